# revision 6
# baseline (speedup 1.0000x reference)
"""Causal attention (B=4, S=4096, D=768) on 8 Trainium2 NeuronCores.

Sharding: zigzag query-strip packing. Each batch b is handled by two cores
(roles): role 0 owns query strips {0,2,5,7}, role 1 owns {1,3,4,6} (strips of
512 rows). Both roles run the IDENTICAL program (SPMD): 4 query supers of 512
rows, with per-super key-block loop bounds TSLOT=(8,16,24,32) 128-row blocks.
Strip->slot assignment is chosen so each role's strip needs <= the slot bound;
the overshoot plus the causal diagonal are killed by additive mask planes
(host-precomputed per role, supplied as input data).

Algebraic refactor vs the naive pipeline: K and V are never materialized.
  scores = (x Wq)(x Wk)^T = (x M) x^T with M = Wq Wk^T precomputed on host,
  so K^T is the raw xkT input and only q' = xq M is projected on device.
  out   = softmax(scores) (x Wv) = (softmax(scores) x) Wv, so the PV matmul
  contracts weights against raw x rows (xr input), and a single small
  U @ Wv projection per query slice finishes the job. This removes the K
  and V projections entirely (the dominant duplicated work across the
  role pair). The softmax denominator comes from an all-ones stationary
  matrix: ones^T @ expT accumulated over key blocks replicates sum_k exp
  into every partition, so normalization of U needs no transpose.
Softmax uses no max-subtraction (scores/sqrt(D) ~ N(0,1); exp is safe in
fp32). Host prep: cast to bf16, transpose x, pack query strips, one 768^3
matmul for M (layout-level work; all O(S^2 D) FLOPs on device).
"""

import math

import numpy as np
import ml_dtypes

P = 128
NEG = -1e9
bf16 = ml_dtypes.bfloat16

# Full-size problem geometry (hardcoded; kernel.py must be self-contained).
B, S, D = 4, 4096, 768
SUP = 512
NSLOT = 4
NQ = NSLOT * SUP
TSLOT = (8, 16, 24, 32)
MASK_KB = 8
ROLE_STRIPS = ((0, 2, 5, 7), (1, 3, 4, 6))
N_CORES = 8


def build_program(S, D, SUP, TSLOT, MASK_KB, out_dtype_np=np.float32):
    """Build the single SPMD Bass program (one core's view).

    Inputs (per core): xkT bf16 [D,S], xqT bf16 [D,NQ], xr bf16 [S,D],
    m bf16 [D,D] (= Wq Wk^T), wv bf16 [D,D], rmask f32 [NSLOT,P,MASK_KB*SUP].
    Output: out f32 [NQ, D] (slot-packed query rows).
    """
    import concourse.bass as bass
    import concourse.tile as tile
    import concourse.mybir as mybir
    from concourse import bacc

    DC = D // P
    NSLOT_ = len(TSLOT)
    NQ_ = NSLOT_ * SUP
    NKB = S // P
    SCALE = 1.0 / math.sqrt(float(D))
    f32 = mybir.dt.float32
    b16 = mybir.dt.bfloat16
    Tmax = max(TSLOT)

    nc = bacc.Bacc("TRN2", target_bir_lowering=False, debug=False)

    xkT = nc.dram_tensor("xkT", [D, S], b16, kind="ExternalInput").ap()
    xqT = nc.dram_tensor("xqT", [D, NQ_], b16, kind="ExternalInput").ap()
    xr = nc.dram_tensor("xr", [S, D], b16, kind="ExternalInput").ap()
    mw = nc.dram_tensor("m", [D, D], b16, kind="ExternalInput").ap()
    wvw = nc.dram_tensor("wv", [D, D], b16, kind="ExternalInput").ap()
    rmask = nc.dram_tensor(
        "rmask", [NSLOT_, P, MASK_KB * SUP], f32, kind="ExternalInput"
    ).ap()
    out = nc.dram_tensor(
        "out", [NQ_, D], mybir.dt.from_np(np.dtype(out_dtype_np)), kind="ExternalOutput"
    ).ap()

    with tile.TileContext(nc) as tc:
        with tc.tile_pool(name="persist", bufs=1) as persist:
            # persistent SBUF tensors
            XKT = persist.tile([P, DC, S], b16, name="XKT")      # x^T (== K^T)
            XR = persist.tile([P, NKB, D], b16, name="XR")       # x by key-block
            QT = persist.tile([P, DC, NQ_], b16, name="QT")      # q'^T = (xq M)^T
            MW = persist.tile([P, DC, D], b16, name="MW")        # M = Wq Wk^T
            WV = persist.tile([P, DC, D], b16, name="WV")        # Wv
            ONES = persist.tile([P, P], b16, name="ONES")
            nc.vector.memset(ONES, 1.0)
            nc.sync.dma_start(XKT, xkT.rearrange("(c p) s -> p c s", p=P))
            nc.sync.dma_start(XR, xr.rearrange("(kb p) d -> p kb d", p=P))
            nc.sync.dma_start(MW, mw.rearrange("(c p) e -> p c e", p=P))
            nc.sync.dma_start(WV, wvw.rearrange("(c p) e -> p c e", p=P))

            # ---------------- phase A: q' projection ----------------
            with (
                tc.tile_pool(name="xstage", bufs=3) as xstage,
                tc.tile_pool(name="ppsum", bufs=4, space="PSUM") as ppsum,
            ):
                xqT_r = xqT.rearrange("(c p) s -> p c s", p=P)
                for ch in range(NQ_ // SUP):
                    xq_t = xstage.tile([P, DC, SUP], b16, tag="xq", name="xq_t")
                    nc.sync.dma_start(
                        xq_t, xqT_r[:, :, ch * SUP:(ch + 1) * SUP]
                    )
                    for do in range(DC):
                        ps = ppsum.tile([P, SUP], f32, tag="proj", name="proj_ps")
                        for dc in range(DC):
                            nc.tensor.matmul(
                                ps,
                                lhsT=MW[:, dc, do * P:(do + 1) * P],
                                rhs=xq_t[:, dc, :],
                                start=(dc == 0),
                                stop=(dc == DC - 1),
                            )
                        nc.any.tensor_copy(
                            out=QT[:, do, ch * SUP:(ch + 1) * SUP], in_=ps
                        )

            # ---------------- phases B-D: attention ----------------
            with (
                tc.tile_pool(name="expp", bufs=1) as expp,
                tc.tile_pool(name="upool", bufs=2) as upool,
                tc.tile_pool(name="mpool", bufs=2) as mpool,
                tc.tile_pool(name="opool", bufs=2) as opool,
                tc.tile_pool(name="spsum", bufs=3, space="PSUM") as spsum,
                tc.tile_pool(name="upsum", bufs=2, space="PSUM") as upsum,
                tc.tile_pool(name="dpsum", bufs=1, space="PSUM") as dpsum,
                tc.tile_pool(name="opsum", bufs=1, space="PSUM") as opsum,
            ):
                expT = expp.tile([P, Tmax, SUP], b16, name="expT")

                # causal wedge trim: for the last 3 key blocks of each super,
                # queries below `off` are masked for both roles, so the
                # score/exp/U matmuls shrink to [off:SUP]. The trimmed expT
                # region is first-touched trimmed (later supers rewrite those
                # kb full-width), so zero it exactly once up front.
                def off_of(T, kb):
                    return P * max(0, kb - (T - NSLOT_))

                for t in range(NSLOT_):
                    T = TSLOT[t]
                    for kb in range(T):
                        off = off_of(T, kb)
                        if off:
                            nc.vector.memset(expT[:, kb, :off], 0.0)

                for t in range(NSLOT_):
                    T = TSLOT[t]
                    q0 = t * SUP
                    # -- B: scores + exp for all key blocks of this super --
                    for kb in range(T):
                        off = off_of(T, kb)
                        ps = spsum.tile([P, SUP], f32, tag="sc", name="sc_ps")
                        for dc in range(DC):
                            nc.tensor.matmul(
                                ps[:, off:],
                                lhsT=XKT[:, dc, kb * P:(kb + 1) * P],
                                rhs=QT[:, dc, q0 + off:q0 + SUP],
                                start=(dc == 0),
                                stop=(dc == DC - 1),
                            )
                        if kb >= T - MASK_KB:
                            kbi = kb - (T - MASK_KB)
                            m = mpool.tile([P, SUP], f32, tag="m", name="m_t")
                            nc.sync.dma_start(
                                m, rmask[t, :, kbi * SUP:(kbi + 1) * SUP]
                            )
                            nc.vector.tensor_add(
                                ps[:, off:], ps[:, off:], m[:, off:]
                            )
                        nc.scalar.activation(
                            expT[:, kb, off:], ps[:, off:],
                            mybir.ActivationFunctionType.Exp, scale=SCALE,
                        )
                    # -- C: denominator first (so recip overlaps U passes),
                    #       then U^T = x^T @ w^T per d-chunk with the
                    #       normalization fused into the PSUM->SBUF copy --
                    psd = dpsum.tile([P, SUP], f32, tag="den", name="den_ps")
                    for kb in range(T):
                        off = off_of(T, kb)
                        nc.tensor.matmul(
                            psd[:, off:] if kb else psd,
                            lhsT=ONES,
                            rhs=expT[:, kb, off:],
                            start=(kb == 0),
                            stop=(kb == T - 1),
                        )
                    recip = upool.tile([P, SUP], f32, tag="recip", name="recip_t")
                    nc.vector.reciprocal(recip, psd)
                    U = upool.tile([P, DC, SUP], b16, tag="u", name="u_t")
                    for dc in range(DC):
                        psu = upsum.tile([P, SUP], f32, tag="ut", name="ut_ps")
                        for kb in range(T):
                            off = off_of(T, kb)
                            nc.tensor.matmul(
                                psu[:, off:] if kb else psu,
                                lhsT=XR[:, kb, dc * P:(dc + 1) * P],
                                rhs=expT[:, kb, off:],
                                start=(kb == 0),
                                stop=(kb == T - 1),
                            )
                        nc.vector.tensor_mul(U[:, dc, :], psu, recip)
                    # -- D: out = (U/den)^T @ Wv per 128-row query slice --
                    for sl in range(SUP // P):
                        pso = opsum.tile([P, D], f32, tag="o", name="o_ps")
                        for (e0, e1) in ((0, 512), (512, D)):
                            for dc in range(DC):
                                nc.tensor.matmul(
                                    pso[:, e0:e1],
                                    lhsT=U[:, dc, sl * P:(sl + 1) * P],
                                    rhs=WV[:, dc, e0:e1],
                                    start=(dc == 0),
                                    stop=(dc == DC - 1),
                                )
                        ot = opool.tile(
                            [P, D], mybir.dt.from_np(np.dtype(out_dtype_np)),
                            tag="ot", name="ot_t",
                        )
                        nc.any.tensor_copy(out=ot, in_=pso)
                        nc.sync.dma_start(
                            out[q0 + sl * P: q0 + (sl + 1) * P, :], ot
                        )

    nc.compile()
    return nc


def make_rmask(role_strips, TSLOT, SUP, MASK_KB):
    nslot = len(TSLOT)
    m = np.zeros((nslot, P, MASK_KB * SUP), np.float32)
    i = np.arange(P)[:, None]
    j = np.arange(SUP)[None, :]
    for t in range(nslot):
        q0 = SUP * role_strips[t]
        T = TSLOT[t]
        for kbi in range(MASK_KB):
            k0 = P * (T - MASK_KB + kbi)
            m[t, :, kbi * SUP:(kbi + 1) * SUP] = np.where(
                q0 + j >= k0 + i, 0.0, NEG
            )
    return m


_nc_cache = {}
last_run = None


def _get_nc():
    key = (S, D, SUP, TSLOT, MASK_KB)
    if key not in _nc_cache:
        _nc_cache[key] = build_program(S, D, SUP, TSLOT, MASK_KB)
    return _nc_cache[key]


def make_in_maps(x, w_b):
    rmasks = [make_rmask(ROLE_STRIPS[r], TSLOT, SUP, MASK_KB) for r in range(2)]
    in_maps = []
    for c in range(N_CORES):
        b, role = c % B, c // B
        xb = x[b].astype(bf16)
        xq = np.concatenate(
            [xb[SUP * s:SUP * (s + 1)] for s in ROLE_STRIPS[role]], axis=0
        )
        in_maps.append({
            "xkT": np.ascontiguousarray(xb.T),
            "xqT": np.ascontiguousarray(xq.T),
            "xr": xb,
            "rmask": rmasks[role],
            **w_b,
        })
    return in_maps


def kernel(x, Wq, Wk, Wv):
    from concourse import bass_utils

    x = np.asarray(x, dtype=np.float32)
    m = np.asarray(Wq, np.float32) @ np.asarray(Wk, np.float32).T
    w_b = {
        "m": m.astype(bf16),
        "wv": np.asarray(Wv, np.float32).astype(bf16),
    }

    nc = _get_nc()

    in_maps = make_in_maps(x, w_b)

    global last_run
    last_run = bass_utils.run_bass_kernel_spmd(
        nc, in_maps, core_ids=list(range(N_CORES))
    )
    res = last_run.results

    out = np.empty((B, S, D), np.float32)
    for c in range(N_CORES):
        b, role = c % B, c // B
        packed = res[c]["out"]
        for t, s in enumerate(ROLE_STRIPS[role]):
            out[b, SUP * s:SUP * (s + 1)] = packed[SUP * t:SUP * (t + 1)]
    return out


if __name__ == "__main__":
    import reference

    inputs = {k: np.asarray(v) for k, v in reference.setup_inputs().items()}
    expected = np.asarray(reference.reference(**inputs))
    actual = kernel(**inputs)
    err = np.abs(actual - expected).max()
    print(f"absmax err: {err:.3e}  rel: {err / np.abs(expected).max():.3e}")
